# revision 12
# baseline (speedup 1.0000x reference)
"""AttentionFlowLayer (BiDAF-style) Trainium2 kernel, 8 NeuronCores.

Sharding: data-parallel over batch N=16 -> 2 batches per core, weights
replicated, no collectives.

Math per batch (Lc=2048, Lq=256, D=256), per 128-row context tile:
  psum S'[i,j] = sum_d c[i,d]*w_m[d]*q[j,d] + qw[j]   (bf16 matmul, f32 psum)
  psum col 256  = cw[i] = c_i . w_c                    (extra rhs column)
  Ap = exp(S' + qw) incl. col 256 = exp(cw)            (ScalarE, no bias)
  m0[i] = rowmax(Ap[:, 0:256]);  eb[i] = m0 * exp(cw)  (q2c numerator;
      the missing cw in Ap cancels in the c2q softmax)
  c2q psum = A' @ [q | 1] -> cols 0..255 = A'@q, col 256 = Z_i (row sum)
  c2q = (A' @ q) / Z_i
  q2c = (sum_i eb_i * c16[i,:]) / sum_i eb_i           (matmul accumulation)
  G tile = [c, c2q, c*c2q, c*q2c] in bf16, host upcasts to f32.

Emission is phase-major across the 16 context tiles of a batch so each
engine sees long runs of back-to-back ops.  Inputs ride the ACT hwdge
ring, outputs the SP ring, so batch 1 loads overlap batch 0 stores.
GpSimd is avoided for element-wise work (it locks SBUF ports against DVE).
"""

import numpy as np

N, LC, LQ, D = 16, 2048, 256, 256
NCORES = 8
NB = N // NCORES      # batches per core
P = 128
T = LC // P           # context tiles per batch
JT = LQ // P          # query partition tiles
DC = D // P           # d chunks
OG = 4                # tiles per output DMA group / elementwise batch

_cache = {}


def _build():
    import concourse.mybir as mybir
    from concourse import bacc
    from concourse.tile import TileContext
    from concourse.masks import make_identity

    f32 = mybir.dt.float32
    bf16 = mybir.dt.bfloat16
    EXP = mybir.ActivationFunctionType.Exp
    COPY = mybir.ActivationFunctionType.Copy
    AX = mybir.AxisListType.X

    nc = bacc.Bacc("TRN2")
    c_in = nc.dram_tensor("emb_context", (NB, LC, D), f32, kind="ExternalInput")
    q_in = nc.dram_tensor("emb_query", (NB, LQ, D), f32, kind="ExternalInput")
    w_in = nc.dram_tensor("W", (3 * D,), f32, kind="ExternalInput")
    out = nc.dram_tensor("out", (NB, LC, 4 * D), bf16, kind="ExternalOutput")

    with TileContext(nc) as tc:
        with (
            tc.tile_pool(name="const", bufs=1) as constp,
            tc.tile_pool(name="qpool", bufs=2) as qpool,
            tc.tile_pool(name="cfull", bufs=2) as cfp,
            tc.tile_pool(name="perb", bufs=2) as perb,
            tc.tile_pool(name="gbig", bufs=2) as gp,
            tc.tile_pool(name="small", bufs=8) as smallp,
            tc.tile_pool(name="ps_s", bufs=3, space="PSUM") as ps_s,
            tc.tile_pool(name="ps_t", bufs=3, space="PSUM") as ps_t,
            tc.tile_pool(name="ps_cq", bufs=2, space="PSUM") as ps_cq,
        ):
            ident = constp.tile([P, P], bf16, tag="ident")
            make_identity(nc, ident)
            ones_row = constp.tile([1, P], bf16, tag="ones_row")
            nc.vector.memset(ones_row, 1.0)
            ones_col = constp.tile([P, 1], bf16, tag="ones_col")
            nc.vector.memset(ones_col, 1.0)
            # W columns: [wc0 wc1 wq0 wq1 wm0 wm1], chunk c covers d=c*128..c*128+127
            wcols = constp.tile([P, 6], f32, tag="wcols")
            nc.scalar.dma_start(wcols, w_in[:].rearrange("(c p) -> p c", p=P))
            wq16 = constp.tile([P, 2], bf16, tag="wq16")
            nc.vector.tensor_copy(wq16, wcols[:, 2:4])

            # ---- all input loads up-front on the ACT hwdge ring ----
            qfs, cfulls = [], []
            for b in range(NB):
                qf = qpool.tile([P, JT, D], f32, tag="qf")
                nc.scalar.dma_start(qf, q_in[b].rearrange("(jt p) d -> p jt d", p=P))
                qfs.append(qf)
            for b in range(NB):
                cfull = cfp.tile([P, T, D], f32, tag="cfull")
                c_r = c_in[b].rearrange("(t p) d -> p t d", p=P)
                CQ = T // 4
                for i in range(4):
                    nc.scalar.dma_start(
                        cfull[:, i * CQ:(i + 1) * CQ, :], c_r[:, i * CQ:(i + 1) * CQ, :]
                    )
                cfulls.append(cfull)

            # ---- PE warm-up burst (~4us) while input DMAs stream:
            # sustained matmul activity flips the HAM clock gate to 2.4 GHz
            # before the real matmuls start.
            warm_ps = ps_cq.tile([P, D + 1], f32, tag="cq")
            for i in range(18):
                nc.tensor.matmul(
                    warm_ps[:, 0:P], lhsT=ident, rhs=ident,
                    start=(i == 0), stop=(i == 17),
                )

            for b in range(NB):
                qf = qfs[b]
                cfull = cfulls[b]
                # q16x: bf16 queries with a ones column (Z accumulator)
                q16x = qpool.tile([P, JT, D + 1], bf16, tag="q16x")
                nc.vector.tensor_copy(q16x[:, :, 0:D], qf)
                nc.vector.memset(q16x[:, :, D:D + 1], 1.0)
                # qT16[p, c, j] = q16[j, c*128+p]
                qT16 = qpool.tile([P, DC, LQ], bf16, tag="qT16")
                for c in range(DC):
                    pst = ps_t.tile([P, LQ], bf16, tag="pst")
                    for jt in range(JT):
                        nc.tensor.transpose(
                            pst[:, jt * P:(jt + 1) * P],
                            q16x[:, jt, c * P:(c + 1) * P],
                            ident,
                        )
                    nc.vector.tensor_copy(qT16[:, c, :], pst)
                # qmTx[:, c, 0:LQ] = qT16 * w_m[c];  col LQ = w_c[c]
                qmTx = qpool.tile([P, DC, LQ + 1], bf16, tag="qmTx")
                for c in range(DC):
                    nc.vector.tensor_scalar_mul(
                        qmTx[:, c, 0:LQ], qT16[:, c, :], wcols[:, 4 + c:5 + c]
                    )
                    nc.vector.tensor_copy(qmTx[:, c, LQ:LQ + 1], wcols[:, c:c + 1])
                # qw row: qw[j] = q_j . w_q ; col LQ stays 0
                ps_qw = ps_s.tile([1, LQ], f32, tag="ps_s")
                for c in range(DC):
                    nc.tensor.matmul(
                        ps_qw,
                        lhsT=wq16[:, c:c + 1],
                        rhs=qT16[:, c, :],
                        start=(c == 0),
                        stop=(c == DC - 1),
                    )
                qwx = qpool.tile([1, LQ + 1], bf16, tag="qwx")
                nc.vector.memset(qwx, 0.0)
                nc.vector.tensor_copy(qwx[:, 0:LQ], ps_qw)

                # per-batch staging / stats (all resident for the batch)
                g012 = gp.tile([P, T, 4 * D], bf16, tag="g012")
                m0 = perb.tile([P, T], bf16, tag="m0")
                cT16 = perb.tile([P, T, D], bf16, tag="ct16")
                Ap = perb.tile([P, T, LQ + 1], bf16, tag="ap")
                ApT = perb.tile([P, T, LQ], bf16, tag="apt")
                invZ = perb.tile([P, T], f32, tag="invz")

                out_r = out[b].rearrange("(t p) d -> p t d", p=P)
                # ---- phase A: cast c -> bf16 (chunk0, batched) + transposes ----
                for t0 in range(0, T, OG):
                    nc.vector.tensor_copy(
                        g012[:, t0:t0 + OG, 0:D], cfull[:, t0:t0 + OG, :]
                    )
                    for t in range(t0, t0 + OG):
                        pst = ps_t.tile([P, D], bf16, tag="pst")
                        for c in range(DC):
                            nc.tensor.transpose(
                                pst[:, c * P:(c + 1) * P],
                                g012[:, t, c * P:(c + 1) * P],
                                ident,
                            )
                        nc.vector.tensor_copy(cT16[:, t, :], pst)
                    # chunk0 is final as soon as the cast lands: store it now
                    nc.sync.dma_start(
                        out_r[:, t0:t0 + OG, 0:D], g012[:, t0:t0 + OG, 0:D]
                    )

                # ---- phase B: S matmuls + cw + exp ----
                for t in range(T):
                    ps_S_t = ps_s.tile([P, LQ + 1], f32, tag="ps_s")
                    for c in range(DC):
                        nc.tensor.matmul(
                            ps_S_t,
                            lhsT=cT16[:, t, c * P:(c + 1) * P],
                            rhs=qmTx[:, c, :],
                            start=(c == 0),
                            stop=False,
                        )
                    nc.tensor.matmul(
                        ps_S_t, lhsT=ones_row, rhs=qwx, start=False, stop=True
                    )
                    nc.scalar.activation(Ap[:, t, :], ps_S_t, EXP)

                # ---- phase C: rowmax (batched) + A' transpose ----
                for t0 in range(0, T, OG):
                    nc.vector.reduce_max(
                        m0[:, t0:t0 + OG], Ap[:, t0:t0 + OG, 0:LQ], axis=AX
                    )
                    for t in range(t0, t0 + OG):
                        psa = ps_t.tile([P, LQ], bf16, tag="pst")
                        for jt in range(JT):
                            nc.tensor.transpose(
                                psa[:, jt * P:(jt + 1) * P],
                                Ap[:, t, jt * P:(jt + 1) * P],
                                ident,
                            )
                        if t % 2 == 0:
                            nc.scalar.copy(ApT[:, t, :], psa)
                        else:
                            nc.vector.tensor_copy(ApT[:, t, :], psa)

                # ---- phase F: q2c (needs only m0 + exp(cw) + chunk0; overlaps D/E) ----
                eb16 = perb.tile([P, T], bf16, tag="eb16")
                nc.vector.tensor_mul(eb16, m0, Ap[:, :, LQ])
                ebrow = smallp.tile([P, 1], f32, tag="ebrow")
                nc.vector.reduce_sum(ebrow, eb16, axis=AX)
                ebrow16 = smallp.tile([P, 1], bf16, tag="ebrow16")
                nc.vector.tensor_copy(ebrow16, ebrow)
                ps_zb = ps_s.tile([1, 1], f32, tag="ps_s")
                nc.tensor.matmul(ps_zb, lhsT=ebrow16, rhs=ones_col, start=True, stop=True)
                zb = smallp.tile([1, 1], f32, tag="zb")
                nc.vector.tensor_copy(zb, ps_zb)
                inv_zb = smallp.tile([1, 1], f32, tag="invzb")
                nc.vector.reciprocal(inv_zb, zb)
                ps_q2c = ps_s.tile([1, D], f32, tag="ps_s")
                for t in range(T):
                    nc.tensor.matmul(
                        ps_q2c,
                        lhsT=eb16[:, t:t + 1],
                        rhs=g012[:, t, 0:D],
                        start=(t == 0),
                        stop=(t == T - 1),
                    )
                q2cn16 = smallp.tile([1, D], bf16, tag="q2cn")
                nc.scalar.activation(q2cn16, ps_q2c, COPY, scale=inv_zb)
                ps_bc = ps_cq.tile([P, D], f32, tag="cq")
                nc.tensor.matmul(ps_bc, lhsT=ones_row, rhs=q2cn16, start=True, stop=True)
                q2cb16 = perb.tile([P, D], bf16, tag="q2cb")
                nc.vector.tensor_copy(q2cb16, ps_bc)

                # ---- phase D: c2q matmuls + normalize; store chunk1 per group ----
                for t in range(T):
                    ps_c2q_t = ps_cq.tile([P, D + 1], f32, tag="cq")
                    for jt in range(JT):
                        nc.tensor.matmul(
                            ps_c2q_t,
                            lhsT=ApT[:, t, jt * P:(jt + 1) * P],
                            rhs=q16x[:, jt, :],
                            start=(jt == 0),
                            stop=(jt == JT - 1),
                        )
                    nc.vector.reciprocal(invZ[:, t:t + 1], ps_c2q_t[:, D:D + 1])
                    nc.scalar.activation(
                        g012[:, t, D:2 * D], ps_c2q_t[:, 0:D], COPY,
                        scale=invZ[:, t:t + 1],
                    )
                    if t % OG == OG - 1:
                        t0 = t - (OG - 1)
                        nc.sync.dma_start(
                            out_r[:, t0:t + 1, D:2 * D], g012[:, t0:t + 1, D:2 * D]
                        )
                        # product chunks for this group, then one merged store:
                        # overlaps the remaining c2q matmuls instead of
                        # serializing after them.
                        nc.vector.tensor_mul(
                            g012[:, t0:t + 1, 2 * D:3 * D],
                            g012[:, t0:t + 1, 0:D],
                            g012[:, t0:t + 1, D:2 * D],
                        )
                        nc.vector.tensor_mul(
                            g012[:, t0:t + 1, 3 * D:4 * D],
                            g012[:, t0:t + 1, 0:D],
                            q2cb16[:, None, :].to_broadcast((P, OG, D)),
                        )
                        nc.sync.dma_start(
                            out_r[:, t0:t + 1, 2 * D:4 * D],
                            g012[:, t0:t + 1, 2 * D:4 * D],
                        )

    nc.compile()
    return nc


def _get_nc():
    if "nc" not in _cache:
        _cache["nc"] = _build()
    return _cache["nc"]


def run(emb_context, emb_query, W, trace=False, **kwargs):
    from concourse.bass_utils import run_bass_kernel_spmd

    nc = _get_nc()
    emb_context = np.asarray(emb_context, dtype=np.float32)
    emb_query = np.asarray(emb_query, dtype=np.float32)
    W = np.asarray(W, dtype=np.float32)
    in_maps = [
        {
            "emb_context": np.ascontiguousarray(emb_context[c * NB:(c + 1) * NB]),
            "emb_query": np.ascontiguousarray(emb_query[c * NB:(c + 1) * NB]),
            "W": W,
        }
        for c in range(NCORES)
    ]
    res = run_bass_kernel_spmd(
        nc, in_maps, core_ids=list(range(NCORES)), trace=trace, **kwargs
    )
    outs = [np.asarray(r["out"], dtype=np.float32) for r in res.results]
    return np.concatenate(outs, axis=0), res


def kernel(emb_context, emb_query, W):
    out, _ = run(emb_context, emb_query, W, trace=False)
    return out



# revision 16
# speedup vs baseline: 1.0465x; 1.0465x over previous
"""AttentionFlowLayer (BiDAF-style) Trainium2 kernel, 8 NeuronCores.

Sharding: data-parallel over batch N=16 -> 2 batches per core, weights
replicated, no collectives.

Math per batch (Lc=2048, Lq=256, D=256), per 128-row context tile:
  psum S'[i,j] = sum_d c[i,d]*w_m[d]*q[j,d] + qw[j]   (bf16 matmul, f32 psum)
  psum col 256  = cw[i] = c_i . w_c                    (extra rhs column)
  Ap = exp(S' + qw) incl. col 256 = exp(cw)            (ScalarE, no bias)
  m0[i] = rowmax(Ap[:, 0:256]);  eb[i] = m0 * exp(cw)  (q2c numerator;
      the missing cw in Ap cancels in the c2q softmax)
  c2q psum = A' @ [q | 1] -> cols 0..255 = A'@q, col 256 = Z_i (row sum)
  c2q = (A' @ q) / Z_i
  q2c = (sum_i eb_i * c16[i,:]) / sum_i eb_i           (matmul accumulation)
  G tile = [c, c2q, c*c2q, c*q2c] in bf16, host upcasts to f32.

Emission is phase-major across the 16 context tiles of a batch so each
engine sees long runs of back-to-back ops.  Inputs ride the ACT hwdge
ring, outputs the SP ring, so batch 1 loads overlap batch 0 stores.
GpSimd is avoided for element-wise work (it locks SBUF ports against DVE).
"""

import numpy as np

N, LC, LQ, D = 16, 2048, 256, 256
NCORES = 8
NB = N // NCORES      # batches per core
P = 128
T = LC // P           # context tiles per batch
JT = LQ // P          # query partition tiles
DC = D // P           # d chunks
OG = 4                # tiles per output DMA group / elementwise batch

_cache = {}


def _build():
    import concourse.mybir as mybir
    from concourse import bacc
    from concourse.tile import TileContext
    from concourse.masks import make_identity

    f32 = mybir.dt.float32
    bf16 = mybir.dt.bfloat16
    EXP = mybir.ActivationFunctionType.Exp
    COPY = mybir.ActivationFunctionType.Copy
    AX = mybir.AxisListType.X

    nc = bacc.Bacc("TRN2")
    c_in = nc.dram_tensor("emb_context", (NB, LC, D), f32, kind="ExternalInput")
    q_in = nc.dram_tensor("emb_query", (NB, LQ, D), f32, kind="ExternalInput")
    w_in = nc.dram_tensor("W", (3 * D,), f32, kind="ExternalInput")
    out = nc.dram_tensor("out", (NB, LC, 4 * D), bf16, kind="ExternalOutput")

    with TileContext(nc) as tc:
        with (
            tc.tile_pool(name="const", bufs=1) as constp,
            tc.tile_pool(name="qpool", bufs=2) as qpool,
            tc.tile_pool(name="cfull", bufs=2) as cfp,
            tc.tile_pool(name="perb", bufs=2) as perb,
            tc.tile_pool(name="gbig", bufs=2) as gp,
            tc.tile_pool(name="small", bufs=8) as smallp,
            tc.tile_pool(name="ps_s", bufs=3, space="PSUM") as ps_s,
            tc.tile_pool(name="ps_t", bufs=3, space="PSUM") as ps_t,
            tc.tile_pool(name="ps_cq", bufs=2, space="PSUM") as ps_cq,
        ):
            ident = constp.tile([P, P], bf16, tag="ident")
            make_identity(nc, ident)
            ones_row = constp.tile([1, P], bf16, tag="ones_row")
            nc.vector.memset(ones_row, 1.0)
            ones_col = constp.tile([P, 1], bf16, tag="ones_col")
            nc.vector.memset(ones_col, 1.0)
            # W columns: [wc0 wc1 wq0 wq1 wm0 wm1], chunk c covers d=c*128..c*128+127
            wcols = constp.tile([P, 6], f32, tag="wcols")
            nc.scalar.dma_start(wcols, w_in[:].rearrange("(c p) -> p c", p=P))
            wq16 = constp.tile([P, 2], bf16, tag="wq16")
            nc.vector.tensor_copy(wq16, wcols[:, 2:4])

            # ---- all input loads up-front on the ACT hwdge ring ----
            qfs, cfulls = [], []
            for b in range(NB):
                qf = qpool.tile([P, JT, D], f32, tag="qf")
                nc.scalar.dma_start(qf, q_in[b].rearrange("(jt p) d -> p jt d", p=P))
                qfs.append(qf)
            for b in range(NB):
                cfull = cfp.tile([P, T, D], f32, tag="cfull")
                c_r = c_in[b].rearrange("(t p) d -> p t d", p=P)
                CQ = T // 4
                for i in range(4):
                    nc.scalar.dma_start(
                        cfull[:, i * CQ:(i + 1) * CQ, :], c_r[:, i * CQ:(i + 1) * CQ, :]
                    )
                cfulls.append(cfull)

            # ---- PE warm-up burst (~4us) while input DMAs stream:
            # sustained matmul activity flips the HAM clock gate to 2.4 GHz
            # before the real matmuls start.
            warm_ps = ps_cq.tile([P, D + 1], f32, tag="cq")
            for i in range(18):
                nc.tensor.matmul(
                    warm_ps[:, 0:P], lhsT=ident, rhs=ident,
                    start=(i == 0), stop=(i == 17),
                )

            for b in range(NB):
                qf = qfs[b]
                cfull = cfulls[b]
                # q16x: bf16 queries with a ones column (Z accumulator)
                q16x = qpool.tile([P, JT, D + 1], bf16, tag="q16x")
                nc.vector.tensor_copy(q16x[:, :, 0:D], qf)
                nc.vector.memset(q16x[:, :, D:D + 1], 1.0)
                # qT16[p, c, j] = q16[j, c*128+p]
                qT16 = qpool.tile([P, DC, LQ], bf16, tag="qT16")
                for c in range(DC):
                    pst = ps_t.tile([P, LQ], bf16, tag="pst")
                    for jt in range(JT):
                        nc.tensor.transpose(
                            pst[:, jt * P:(jt + 1) * P],
                            q16x[:, jt, c * P:(c + 1) * P],
                            ident,
                        )
                    nc.vector.tensor_copy(qT16[:, c, :], pst)
                # qmTx[:, c, 0:LQ] = qT16 * w_m[c];  col LQ = w_c[c]
                qmTx = qpool.tile([P, DC, LQ + 1], bf16, tag="qmTx")
                for c in range(DC):
                    nc.vector.tensor_scalar_mul(
                        qmTx[:, c, 0:LQ], qT16[:, c, :], wcols[:, 4 + c:5 + c]
                    )
                    nc.vector.tensor_copy(qmTx[:, c, LQ:LQ + 1], wcols[:, c:c + 1])
                # qw row: qw[j] = q_j . w_q ; col LQ stays 0
                ps_qw = ps_s.tile([1, LQ], f32, tag="ps_s")
                for c in range(DC):
                    nc.tensor.matmul(
                        ps_qw,
                        lhsT=wq16[:, c:c + 1],
                        rhs=qT16[:, c, :],
                        start=(c == 0),
                        stop=(c == DC - 1),
                    )
                qwx = qpool.tile([1, LQ + 1], bf16, tag="qwx")
                nc.vector.memset(qwx, 0.0)
                nc.vector.tensor_copy(qwx[:, 0:LQ], ps_qw)

                # per-batch staging / stats (all resident for the batch)
                g012 = gp.tile([P, T, 4 * D], bf16, tag="g012")
                m0 = perb.tile([P, T], bf16, tag="m0")
                cT16 = perb.tile([P, T, D], bf16, tag="ct16")
                Ap = perb.tile([P, T, LQ + 1], bf16, tag="ap")
                ApT = perb.tile([P, T, LQ], bf16, tag="apt")
                invZ = perb.tile([P, T], f32, tag="invz")

                out_r = out[b].rearrange("(t p) d -> p t d", p=P)
                # Phases A-D run per 8-tile half so half-0's c2q results and
                # stores flow while half-1 is still in its S phase; only q2c
                # (rowmax over all 16 tiles) and the c*q2c products are
                # batch-global.
                for hlo in (0, T // 2):
                    hhi = hlo + T // 2
                    # -- phase A: cast c -> bf16 (chunk0, batched) + transposes
                    for t0 in range(hlo, hhi, OG):
                        nc.vector.tensor_copy(
                            g012[:, t0:t0 + OG, 0:D], cfull[:, t0:t0 + OG, :]
                        )
                        for t in range(t0, t0 + OG):
                            pst = ps_t.tile([P, D], bf16, tag="pst")
                            for c in range(DC):
                                nc.tensor.transpose(
                                    pst[:, c * P:(c + 1) * P],
                                    g012[:, t, c * P:(c + 1) * P],
                                    ident,
                                )
                            nc.vector.tensor_copy(cT16[:, t, :], pst)
                        # chunk0 is final as soon as the cast lands
                        nc.sync.dma_start(
                            out_r[:, t0:t0 + OG, 0:D], g012[:, t0:t0 + OG, 0:D]
                        )
                    # -- phase B: S matmuls + exp (covers the cw column too)
                    for t in range(hlo, hhi):
                        ps_S_t = ps_s.tile([P, LQ + 1], f32, tag="ps_s")
                        for c in range(DC):
                            nc.tensor.matmul(
                                ps_S_t,
                                lhsT=cT16[:, t, c * P:(c + 1) * P],
                                rhs=qmTx[:, c, :],
                                start=(c == 0),
                                stop=False,
                            )
                        nc.tensor.matmul(
                            ps_S_t, lhsT=ones_row, rhs=qwx, start=False, stop=True
                        )
                        nc.scalar.activation(Ap[:, t, :], ps_S_t, EXP)
                    # -- phase C: rowmax (batched) + A' transpose
                    for t0 in range(hlo, hhi, OG):
                        nc.vector.reduce_max(
                            m0[:, t0:t0 + OG], Ap[:, t0:t0 + OG, 0:LQ], axis=AX
                        )
                        for t in range(t0, t0 + OG):
                            psa = ps_t.tile([P, LQ], bf16, tag="pst")
                            for jt in range(JT):
                                nc.tensor.transpose(
                                    psa[:, jt * P:(jt + 1) * P],
                                    Ap[:, t, jt * P:(jt + 1) * P],
                                    ident,
                                )
                            if t % 2 == 0:
                                nc.scalar.copy(ApT[:, t, :], psa)
                            else:
                                nc.vector.tensor_copy(ApT[:, t, :], psa)
                    # -- phase D: c2q + normalize; chunk1/chunk2 per group
                    for t in range(hlo, hhi):
                        ps_c2q_t = ps_cq.tile([P, D + 1], f32, tag="cq")
                        for jt in range(JT):
                            nc.tensor.matmul(
                                ps_c2q_t,
                                lhsT=ApT[:, t, jt * P:(jt + 1) * P],
                                rhs=q16x[:, jt, :],
                                start=(jt == 0),
                                stop=(jt == JT - 1),
                            )
                        nc.vector.reciprocal(invZ[:, t:t + 1], ps_c2q_t[:, D:D + 1])
                        nc.scalar.activation(
                            g012[:, t, D:2 * D], ps_c2q_t[:, 0:D], COPY,
                            scale=invZ[:, t:t + 1],
                        )
                        if t % OG == OG - 1:
                            t0 = t - (OG - 1)
                            nc.sync.dma_start(
                                out_r[:, t0:t + 1, D:2 * D],
                                g012[:, t0:t + 1, D:2 * D],
                            )
                            nc.vector.tensor_mul(
                                g012[:, t0:t + 1, 2 * D:3 * D],
                                g012[:, t0:t + 1, 0:D],
                                g012[:, t0:t + 1, D:2 * D],
                            )
                            nc.sync.dma_start(
                                out_r[:, t0:t + 1, 2 * D:3 * D],
                                g012[:, t0:t + 1, 2 * D:3 * D],
                            )

                # ---- phase F: q2c (rowmax of all 16 tiles is now ready) ----
                eb16 = perb.tile([P, T], bf16, tag="eb16")
                nc.vector.tensor_mul(eb16, m0, Ap[:, :, LQ])
                ebrow = smallp.tile([P, 1], f32, tag="ebrow")
                nc.vector.reduce_sum(ebrow, eb16, axis=AX)
                ebrow16 = smallp.tile([P, 1], bf16, tag="ebrow16")
                nc.vector.tensor_copy(ebrow16, ebrow)
                ps_zb = ps_s.tile([1, 1], f32, tag="ps_s")
                nc.tensor.matmul(ps_zb, lhsT=ebrow16, rhs=ones_col, start=True, stop=True)
                zb = smallp.tile([1, 1], f32, tag="zb")
                nc.vector.tensor_copy(zb, ps_zb)
                inv_zb = smallp.tile([1, 1], f32, tag="invzb")
                nc.vector.reciprocal(inv_zb, zb)
                ps_q2c = ps_s.tile([1, D], f32, tag="ps_s")
                for t in range(T):
                    nc.tensor.matmul(
                        ps_q2c,
                        lhsT=eb16[:, t:t + 1],
                        rhs=g012[:, t, 0:D],
                        start=(t == 0),
                        stop=(t == T - 1),
                    )
                q2cn16 = smallp.tile([1, D], bf16, tag="q2cn")
                nc.scalar.activation(q2cn16, ps_q2c, COPY, scale=inv_zb)
                ps_bc = ps_cq.tile([P, D], f32, tag="cq")
                nc.tensor.matmul(ps_bc, lhsT=ones_row, rhs=q2cn16, start=True, stop=True)
                q2cb16 = perb.tile([P, D], bf16, tag="q2cb")
                nc.vector.tensor_copy(q2cb16, ps_bc)

                # ---- chunk3 (c * q2c): batch-global, per-group muls + stores
                for t0 in range(0, T, OG):
                    nc.vector.tensor_mul(
                        g012[:, t0:t0 + OG, 3 * D:4 * D],
                        g012[:, t0:t0 + OG, 0:D],
                        q2cb16[:, None, :].to_broadcast((P, OG, D)),
                    )
                    nc.sync.dma_start(
                        out_r[:, t0:t0 + OG, 3 * D:4 * D],
                        g012[:, t0:t0 + OG, 3 * D:4 * D],
                    )

    nc.compile()
    return nc


def _get_nc():
    if "nc" not in _cache:
        _cache["nc"] = _build()
    return _cache["nc"]


def run(emb_context, emb_query, W, trace=False, **kwargs):
    from concourse.bass_utils import run_bass_kernel_spmd

    nc = _get_nc()
    emb_context = np.asarray(emb_context, dtype=np.float32)
    emb_query = np.asarray(emb_query, dtype=np.float32)
    W = np.asarray(W, dtype=np.float32)
    in_maps = [
        {
            "emb_context": np.ascontiguousarray(emb_context[c * NB:(c + 1) * NB]),
            "emb_query": np.ascontiguousarray(emb_query[c * NB:(c + 1) * NB]),
            "W": W,
        }
        for c in range(NCORES)
    ]
    res = run_bass_kernel_spmd(
        nc, in_maps, core_ids=list(range(NCORES)), trace=trace, **kwargs
    )
    outs = [np.asarray(r["out"], dtype=np.float32) for r in res.results]
    return np.concatenate(outs, axis=0), res


def kernel(emb_context, emb_query, W):
    out, _ = run(emb_context, emb_query, W, trace=False)
    return out

